# revision 27
# baseline (speedup 1.0000x reference)
"""LITv1 transformer block on 8 TRN2 NeuronCores, data-parallel over batch.

v3 design:
- Attention phase in fp8e4 with DoubleRow matmuls (QKV, S, PV, proj):
  2 k-tiles per instruction at 0.5 cycles/row.
- Relative-position bias folded into the S PSUM accumulation via an fp8
  identity-pair preload matmul (8x bias table, exp scale 0.125, bias -4
  shift keeps exp outputs within fp8 range).
- Softmax denominator via ones-column in V (DR out [65, N]); per-head
  reciprocal + broadcast matmul + normalize multiply.
- LN1 mean/rstd computed host-side (x is an input); avoids Act sqrt
  table switches against exp.
- LN normalize outputs bf16; transposes done by the DMA xbar
  (dma_start_transpose), not the PE.
- MLP in bf16 (fp8 too lossy there); fc1 dff-major + fused gelu,
  fc2 token-major; residual stream r1 kept in SBUF as bf16.
"""
import sys
from contextlib import ExitStack

import numpy as np
import ml_dtypes

sys.path.insert(0, "/opt/trn_rl_repo")

import concourse.bass as bass  # noqa: E402
import concourse.mybir as mybir  # noqa: E402
import concourse.tile as tile  # noqa: E402
from concourse import bacc  # noqa: E402
from concourse.bass_utils import run_bass_kernel_spmd  # noqa: E402

F32 = mybir.dt.float32
BF16 = mybir.dt.bfloat16
F8 = mybir.dt.float8e4
AF = mybir.ActivationFunctionType
ALU = mybir.AluOpType
PM = mybir.MatmulPerfMode

B, N, C = 64, 256, 1024
H, DH = 16, 64
DFF = 4 * C
NCORES = 8
BLOC = B // NCORES          # 8 batches per core
TOK = BLOC * N              # 2048 tokens per core
KC = C // 128               # 8 contraction chunks
NB = 2                      # MLP token blocks
BT = TOK // NB              # 1024 tokens per MLP block

F8NP = ml_dtypes.float8_e4m3
BFNP = ml_dtypes.bfloat16


def build():
    nc = bacc.Bacc("TRN2")
    x_d = nc.dram_tensor("x", [TOK, C], BF16, kind="ExternalInput")
    ms1_d = nc.dram_tensor("ms1", [128, 16, 2], F32, kind="ExternalInput")
    wqkv_d = nc.dram_tensor("wqkv", [128, 4, 2, 3 * C], F8, kind="ExternalInput")
    wproj_d = nc.dram_tensor("wproj", [128, 4, 2, C], F8, kind="ExternalInput")
    btab_d = nc.dram_tensor("btab", [128, 8, 2, 2 * N], F8, kind="ExternalInput")
    wfc1h_d = nc.dram_tensor("wfc1h", [128, 4, 2, DFF], F8, kind="ExternalInput")
    wfc1l_d = nc.dram_tensor("wfc1l", [128, 4, 2, DFF], F8, kind="ExternalInput")
    wfc2h_d = nc.dram_tensor("wfc2h", [128, 16, 2, C], F8, kind="ExternalInput")
    wfc2l_d = nc.dram_tensor("wfc2l", [128, 16, 2, C], F8, kind="ExternalInput")
    y_d = nc.dram_tensor("y", [TOK, C], F32, kind="ExternalOutput")

    with tile.TileContext(nc) as tc:
        with ExitStack() as _es:
            _es.enter_context(nc.allow_low_precision(reason="bf16 residual/LN"))
            consts = _es.enter_context(tc.tile_pool(name="consts", bufs=1))
            # ident2[p, i, k] = 1 if k == 64*i + (p % 64): fp8 ident pair for
            # the bias preload (duplicated in both partition halves).
            id2 = consts.tile([128, 2, 128], F8)
            # build via iota trick: set with 128 tiny memsets would be slow;
            # use affine_select-free approach: write from DRAM instead.
            # (cheap: 32KB one-off DMA)
            id2_d = nc.dram_tensor("id2", [128, 2, 128], F8, kind="ExternalInput")
            nc.sync.dma_start(id2, id2_d[:])
            ones64_bf = consts.tile([1, 64], BF16)
            nc.vector.memset(ones64_bf, 1.0)
            ones8 = consts.tile([128, 2, 128], F8)
            nc.vector.memset(ones8, 1.0)
            eps_sb = consts.tile([128, 1], F32)
            nc.vector.memset(eps_sb, 1e-5)
            nbias = consts.tile([128, 1], F32)
            nc.vector.memset(nbias, -3.5)
            ms1 = consts.tile([128, 16, 2], F32)
            nc.sync.dma_start(ms1, ms1_d[:])
            mv2 = consts.tile([128, 16, 2], F32)
            rstd2 = consts.tile([128, 16], F32)
            srt = consts.tile([128, 16], F32)
            r1_bf = consts.tile([128, 16, C], BF16)
            pbw1 = _es.enter_context(tc.tile_pool(name="pbw1", bufs=2))
            pbw2 = _es.enter_context(tc.tile_pool(name="pbw2", bufs=4))
            pbt2 = _es.enter_context(tc.tile_pool(name="pbt2", bufs=1))
            pbx = _es.enter_context(tc.tile_pool(name="pbx", bufs=1))
            # shared tag: blk0's xn2T is dead once its hi/lo split is made
            # (phase A b4/b5), so blk1's transposes can reuse the buffer
            xn2Ts = [pbt2.tile([128, KC, BT], BF16, tag="xn2T",
                               name=f"xn2T{i}") for i in range(NB)]
            # hi/lo fp8 split of the transposed LN2 output, for 3-product
            # DoubleRow fc1 (Xh@(W1h+W1l) + Xl@W1h).  Shared tags: blk1
            # reuses blk0's buffers (its split is emitted after blk0's fc1
            # reads so the WAR dependency orders correctly).
            XhXl = [
                (pbx.tile([128, KC, BT], F8, tag="Xh", name=f"Xh{b_}"),
                 pbx.tile([128, KC, BT], F8, tag="Xl", name=f"Xl{b_}"))
                for b_ in range(NB)
            ]

            def emit_xsplit(blk, half, pool_eng):
                Xh, Xl = XhXl[blk]
                sl = slice(half * 512, (half + 1) * 512)
                if pool_eng:
                    nc.gpsimd.tensor_copy(Xh[:, :, sl], xn2Ts[blk][:, :, sl])
                    nc.gpsimd.tensor_sub(
                        Xl[:, :, sl], xn2Ts[blk][:, :, sl], Xh[:, :, sl]
                    )
                else:
                    nc.scalar.activation(
                        Xh[:, :, sl], xn2Ts[blk][:, :, sl], AF.Copy
                    )
                    nc.vector.tensor_sub(
                        Xl[:, :, sl], xn2Ts[blk][:, :, sl], Xh[:, :, sl]
                    )

            def emit_wf1(blk, out, s0, s1):
                for s in range(s0, s1):
                    wf1h = pbw1.tile([128, 4, 2, 512], F8, tag="wf1h",
                                     name=f"wf1h_{blk}_{s}")
                    wf1l = pbw1.tile([128, 4, 2, 512], F8, tag="wf1l",
                                     name=f"wf1l_{blk}_{s}")
                    nc.sync.dma_start(
                        wf1h, wfc1h_d[:, :, :, s * 512 : (s + 1) * 512]
                    )
                    nc.sync.dma_start(
                        wf1l, wfc1l_d[:, :, :, s * 512 : (s + 1) * 512]
                    )
                    out.append((wf1h, wf1l))

            mv2c = consts.tile([128, 16, 2], F32)

            def emit_rstd(lo, hi):
                # copy gates the sqrt block on the last producer of mv2[lo:hi]
                # so the scheduler cannot scatter the sqrts (and their Act
                # table loads) across earlier exp batches
                nc.vector.tensor_copy(mv2c[:, lo:hi, :], mv2[:, lo:hi, :])
                for ti in range(lo, hi):
                    nc.scalar.activation(
                        srt[:, ti : ti + 1], mv2c[:, ti, 1:2], AF.Sqrt,
                        bias=eps_sb, scale=1.0,
                    )
                nc.vector.reciprocal(rstd2[:, lo:hi], srt[:, lo:hi])
            wf1s = []

            # ---------------- Phase A: attention + proj ----------------
            with ExitStack() as es:
                tp = lambda nm, bufs, **kw: es.enter_context(tc.tile_pool(name=nm, bufs=bufs, **kw))
                paw = tp("paw", 1); pax = tp("pax", 3); pan = tp("pan", 1)
                pat = tp("pat", 1); pa8 = tp("pa8", 2); paq = tp("paq", 2)
                pav = tp("pav", 2); par = tp("par", 4); pao = tp("pao", 2)
                psQV = tp("psQV", 2, space="PSUM")
                psS = tp("psS", 2, space="PSUM"); psO = tp("psO", 2, space="PSUM")
                psDB = tp("psDB", 2, space="PSUM")
                wqkv_sb = paw.tile([128, 4, 2, 3 * C], F8)
                wproj_sb = paw.tile([128, 4, 2, C], F8)
                btab = paw.tile([128, 8, 2, 2 * N], F8)

                fronts = {}
                oalls = {}

                def emit_front(b):
                    t0 = b * N
                    # --- LN1 (host stats) -> xn bf16 -> DMA transpose ---
                    xt = pax.tile([128, 2, C], BF16, tag="x")
                    xnT = pat.tile([128, KC, N], BF16, tag="xnT")
                    for t in range(2):
                        ti = 2 * b + t
                        nc.sync.dma_start(
                            xt[:, t, :], x_d[t0 + t * 128 : t0 + (t + 1) * 128, :]
                        )
                        xn = pan.tile([128, C], BF16, tag="xn")
                        ln_eng = nc.vector if b == 0 else nc.gpsimd
                        ln_eng.tensor_scalar(
                            xn, xt[:, t, :], ms1[:, ti, 0:1], ms1[:, ti, 1:2],
                            ALU.subtract, ALU.mult,
                        )
                        nc.sync.dma_start_transpose(
                            xnT[:, :, t * 128 : (t + 1) * 128], xn
                        )
                    if b == 0:
                        # weight DMAs via the gpsimd SWDGE queue: Pool is
                        # idle at startup and the SP queue stays free for the
                        # LN-dependent transpose chain; chunked so the first
                        # QKV matmuls only wait for the first 512-col chunk
                        for ci in range(4):
                            nc.gpsimd.dma_start(
                                wqkv_sb[:, :, :, ci * 512 : (ci + 1) * 512],
                                wqkv_d[:, :, :, ci * 512 : (ci + 1) * 512],
                            )
                        nc.gpsimd.dma_start(btab, btab_d[:])
                        nc.gpsimd.dma_start(
                            wqkv_sb[:, :, :, 2 * C :], wqkv_d[:, :, :, 2 * C :]
                        )
                        nc.gpsimd.dma_start(wproj_sb, wproj_d[:])
                    xnT8 = pa8.tile([128, KC, N], F8, tag="xnT8")
                    for t in range(2):
                        nc.gpsimd.tensor_copy(
                            xnT8[:, :, t * 128 : (t + 1) * 128],
                            xnT[:, :, t * 128 : (t + 1) * 128],
                        )

                    # --- QKV --- (b==0: split q/k over token halves so the
                    # first matmuls only wait on the first front half)
                    qkT8 = paq.tile([128, 2 * KC, N], F8, tag="qkT8")
                    tsplit = 2 if b == 0 else 1
                    for co in range(2 * KC):
                        qv = psQV.tile([128, 512], F32, tag="qv")
                        qp = qv[:, 0:N]
                        for ts in range(tsplit):
                            tsl = slice(ts * (N // tsplit),
                                        (ts + 1) * (N // tsplit))
                            for kk in range(4):
                                nc.tensor.matmul(
                                    qp[:, tsl],
                                    wqkv_sb[:, kk, :, co * 128 : (co + 1) * 128],
                                    xnT8.rearrange("p (a i) n -> p a i n", i=2)[
                                        :, kk, :, tsl
                                    ],
                                    start=(kk == 0),
                                    stop=(kk == 3),
                                    perf_mode=PM.DoubleRow,
                                )
                        if co % 2 == 0:
                            nc.vector.tensor_copy(qkT8[:, co, :], qp)
                        else:
                            nc.scalar.copy(qkT8[:, co, :], qp)
                    v8 = pav.tile([128, 2, H, DH], F8, tag="v8")
                    for t in range(2):
                        for vc in range(2):
                            vp = psQV.tile([128, 512], F32, tag="qv")
                            for kk in range(4):
                                nc.tensor.matmul(
                                    vp,
                                    xnT8.rearrange("p (a i) n -> p a i n", i=2)[
                            :, kk, :, t * 128 : (t + 1) * 128
                                    ],
                                    wqkv_sb[:, kk, :, 2 * C + vc * 512 : 2 * C + (vc + 1) * 512],
                                    start=(kk == 0),
                                    stop=(kk == 3),
                                    perf_mode=PM.DoubleRow,
                                )
                            nc.vector.tensor_copy(
                                v8[:, t, vc * 8 : (vc + 1) * 8, :],
                                vp.rearrange("p (h d) -> p h d", h=8),
                            )

                    fronts[b] = (xt, qkT8, v8)

                def emit_attn(b):
                    xt, qkT8, v8 = fronts[b]
                    # --- attention, head-pipelined ---
                    sps, p8s, ops, rds, dbs = {}, {}, {}, {}, {}
                    oall8 = pao.tile([128, KC, N], F8, tag="oall8",
                                     name=f"oall8_{b}")
                    oalls[b] = oall8

                    def emit_S(h):
                        pb = 32 * (h % 4)
                        cp = 2 * (h // 4)
                        hb = 64 * (h // 8)
                        p8 = par.tile([128, 2, N], F8, tag="p8", name=f"p8_{b}_{h}")
                        sp = psS.tile([128, 2, N], F32, tag="sp", name=f"sp_{b}_{h}")
                        spf = sp.rearrange("p a n -> p (a n)")
                        nc.tensor.matmul(
                            spf,
                            id2[hb : hb + 64, :, :],
                            btab[hb : hb + 64, h % 8, :, :],
                            start=True,
                            stop=False,
                            perf_mode=PM.DoubleRow,
                            skip_group_check=True,
                            tile_position=(hb, 0),
                        )
                        for nk in range(2):
                            nc.tensor.matmul(
                                sp[:, nk, :],
                                qkT8[pb : pb + 32, KC + cp : KC + cp + 2,
                                     nk * 128 : (nk + 1) * 128],
                                qkT8[pb : pb + 32, cp : cp + 2, :],
                                start=False,
                                stop=(nk == 1),
                                perf_mode=PM.DoubleRow,
                                skip_group_check=True,
                                tile_position=(pb, 0),
                            )
                        nc.scalar.activation(
                            p8.rearrange("p a n -> p (a n)"), spf, AF.Exp,
                            bias=nbias, scale=0.125,
                        )
                        p8s[h] = p8

                    def emit_PV(h):
                        op = psO.tile([64, N], F32, tag="op", name=f"op_{b}_{h}")
                        nc.tensor.matmul(
                            op,
                            v8[:, :, h, :],
                            p8s[h][:],
                            start=True,
                            stop=True,
                            perf_mode=PM.DoubleRow,
                        )
                        db = psDB.tile([64, 2, N], F32, tag="db", name=f"db_{b}_{h}")
                        nc.tensor.matmul(
                            db[0:1, 0, :], ones8[:, :, 0:1], p8s[h][:],
                            start=True, stop=True, perf_mode=PM.DoubleRow,
                        )
                        rd = par.tile([1, N], BF16, tag="rd", name=f"rd_{b}_{h}")
                        nc.vector.reciprocal(rd, db[0:1, 0, :])
                        ops[h] = op
                        dbs[h] = db
                        rds[h] = rd

                    def emit_norm(h):
                        bc = dbs[h][:, 1, :]
                        nc.tensor.matmul(
                            bc, ones64_bf, rds[h], start=True, stop=True
                        )
                        bc_sb = par.tile([64, N], BF16, tag="bcs", name=f"bcs_{b}_{h}")
                        nc.scalar.copy(bc_sb, bc)
                        nc.vector.tensor_mul(
                            oall8[64 * (h % 2) : 64 * (h % 2) + 64, h // 2, :],
                            ops[h][:],
                            bc_sb,
                        )

                    for h in range(H):
                        emit_S(h)
                        if h >= 1:
                            emit_PV(h - 1)
                        if h >= 2:
                            emit_norm(h - 2)
                    emit_PV(H - 1)
                    emit_norm(H - 2)
                    emit_norm(H - 1)

                def emit_proj(b):
                    xt, qkT8, v8 = fronts[b]
                    oall8 = oalls[b]
                    t0 = b * N
                    # --- proj + residual -> r1_bf, LN2 stats ---
                    for t in range(2):
                        ti = 2 * b + t
                        stats = pan.tile([128, 2, 6], F32, tag="st2")
                        for co in range(2):
                            pp = psQV.tile([128, 512], F32, tag="qv")
                            for kk in range(4):
                                nc.tensor.matmul(
                                    pp,
                                    oall8[:, 2 * kk : 2 * kk + 2,
                              t * 128 : (t + 1) * 128],
                                    wproj_sb[:, kk, :, co * 512 : (co + 1) * 512],
                                    start=(kk == 0),
                                    stop=(kk == 3),
                                    perf_mode=PM.DoubleRow,
                                )
                            nc.vector.tensor_add(
                                r1_bf[:, ti, co * 512 : (co + 1) * 512],
                                pp,
                                xt[:, t, co * 512 : (co + 1) * 512],
                            )
                            nc.vector.bn_stats(
                                stats[:, co, :],
                                r1_bf[:, ti, co * 512 : (co + 1) * 512],
                            )
                        nc.vector.bn_aggr(mv2[:, ti, :], stats)

                    if b == 3:
                        emit_rstd(0, 8)
                        for t in range(BT // 128):
                            xn2 = pan.tile([128, C], BF16, tag="xn2e",
                               name=f"xn2e_{t}")
                            nc.vector.tensor_scalar(
                                xn2, r1_bf[:, t, :], mv2[:, t, 0:1],
                                rstd2[:, t : t + 1], ALU.subtract, ALU.mult,
                            )
                            nc.sync.dma_start_transpose(
                                xn2Ts[0][:, :, t * 128 : (t + 1) * 128], xn2
                            )



                emit_front(0)
                for b in range(BLOC):
                    emit_attn(b)
                    if b + 1 < BLOC:
                        emit_front(b + 1)
                    emit_proj(b)
                    del fronts[b], oalls[b]

            # ------- Phase B: MLP, fp8 DoubleRow with hi/lo 3-product -------
            # fc1: Xh@(W1h+W1l) + Xl@W1h at 8x weight scale (undone in the
            # gelu scale); fc2: Hh@(W2h+W2l) + Hl@W2h at 32x (undone in the
            # fused eviction).  rstd2 for all tiles first (single sqrt-table
            # block on Act).
            with (
                tc.tile_pool(name="pbs", bufs=1) as pbs,
                tc.tile_pool(name="psF1", bufs=4, space="PSUM") as psF1,
                tc.tile_pool(name="psF2", bufs=1, space="PSUM") as psF2,
            ):
                # blk0's first fc1 weight slices before anything else hits
                # the SP queue (stall-free: within pbw1's rotation depth)
                wf1s.clear()
                emit_wf1(0, wf1s, 0, 2)
                emit_rstd(8, 16)
                # blk0 hi/lo split first: it reads xn2Ts[0], whose buffer the
                # blk1 transposes below will reuse (shared tag)
                emit_xsplit(0, 0, pool_eng=False)
                emit_xsplit(0, 1, pool_eng=False)
                for t in range(BT // 128):
                    ti = 8 + t
                    xn2 = pbs.tile([128, C], BF16, tag="xn2p", name=f"xn2p_{t}",
                                   bufs=2)
                    nc.vector.tensor_scalar(
                        xn2, r1_bf[:, ti, :], mv2[:, ti, 0:1],
                        rstd2[:, ti : ti + 1], ALU.subtract, ALU.mult,
                    )
                    nc.sync.dma_start_transpose(
                        xn2Ts[1][:, :, t * 128 : (t + 1) * 128], xn2
                    )
                for blk in range(NB):
                    if blk == 1:
                        emit_xsplit(1, 0, pool_eng=False)
                        emit_xsplit(1, 1, pool_eng=False)
                    with ExitStack() as esb:
                        tpb = lambda nm, bufs, **kw: esb.enter_context(tc.tile_pool(name=nm, bufs=bufs, **kw))
                        pbh = tpb("pbh", 1); pbg = tpb("pbg", 3)
                        Xh, Xl = XhXl[blk]
                        XhV = Xh.rearrange("p (a i) n -> p a i n", i=2)
                        XlV = Xl.rearrange("p (a i) n -> p a i n", i=2)
                        Hh = pbh.tile([128, 32, BT], F8, tag="Hh",
                                      name=f"Hh_{blk}")
                        Hl = pbh.tile([128, 32, BT], F8, tag="Hl",
                                      name=f"Hl_{blk}")
                        HhV = Hh.rearrange("p (a i) n -> p a i n", i=2)
                        HlV = Hl.rearrange("p (a i) n -> p a i n", i=2)
                        if blk > 0:
                            wf1s.clear()
                            emit_wf1(blk, wf1s, 0, 2)
                        for s in range(8):
                            # depth-2 JIT prefetch: never a dep-stalled DMA
                            # parked at the SP queue head for long
                            if s + 2 <= 7:
                                emit_wf1(blk, wf1s, s + 2, s + 3)
                            wf1h, wf1l = wf1s[s]
                            for dc in range(4):
                                ch = s * 4 + dc
                                for th in range(BT // 512):
                                    fp = psF1.tile([128, 512], F32, tag="fp")
                                    k = 0
                                    for W, X in ((wf1h, XhV), (wf1l, XhV),
                                                 (wf1h, XlV)):
                                        for a in range(4):
                                            nc.tensor.matmul(
                                                fp,
                                                W[:, a, :, dc * 128 : (dc + 1) * 128],
                                                X[:, a, :, th * 512 : (th + 1) * 512],
                                                start=(k == 0),
                                                stop=(k == 11),
                                                perf_mode=PM.DoubleRow,
                                            )
                                            k += 1
                                    tsl = slice(th * 512, (th + 1) * 512)
                                    nc.scalar.activation(
                                        Hh[:, ch, tsl], fp,
                                        AF.Gelu_apprx_tanh, scale=0.125,
                                    )
                                    hb = pbg.tile([128, 512], BF16, tag="hb")
                                    nc.scalar.activation(
                                        hb, fp, AF.Gelu_apprx_tanh, scale=0.125,
                                    )
                                    nc.gpsimd.tensor_sub(
                                        Hl[:, ch, tsl], hb, Hh[:, ch, tsl]
                                    )

                        # fc2 token-major + residual -> y (2 token groups
                        # of 4 so psF2 fits in 4 banks alongside psF1)
                        with ExitStack() as esc:
                            tpc = lambda nm, bufs, **kw: esc.enter_context(tc.tile_pool(name=nm, bufs=bufs, **kw))
                            pby = tpc("pby", 2)
                            for co in range(2):
                                wf2s = []
                                for kh in range(4):
                                    w2h = pbw2.tile([128, 4, 2, 512], F8,
                                                    tag="w2h",
                                                    name=f"w2h_{blk}_{co}_{kh}")
                                    w2l = pbw2.tile([128, 4, 2, 512], F8,
                                                    tag="w2l",
                                                    name=f"w2l_{blk}_{co}_{kh}")
                                    nc.sync.dma_start(
                                        w2h,
                                        wfc2h_d[:, kh * 4 : (kh + 1) * 4, :,
                                                co * 512 : (co + 1) * 512],
                                    )
                                    nc.sync.dma_start(
                                        w2l,
                                        wfc2l_d[:, kh * 4 : (kh + 1) * 4, :,
                                                co * 512 : (co + 1) * 512],
                                    )
                                    wf2s.append((w2h, w2l))
                                for tg in range(2):
                                    op2s = [
                                        psF2.tile([128, 512], F32, tag=f"op2_{tq}",
                                                  name=f"op2_{blk}_{co}_{tg}_{tq}")
                                        for tq in range(4)
                                    ]
                                    for kh in range(4):
                                        w2h, w2l = wf2s[kh]
                                        for tq in range(4):
                                            t = tg * 4 + tq
                                            for kk in range(4):
                                                a = kh * 4 + kk
                                                for pi, (Hs, Ws) in enumerate(
                                                    ((HhV, w2h), (HhV, w2l),
                                                     (HlV, w2h))
                                                ):
                                                    nc.tensor.matmul(
                                                        op2s[tq],
                                                        Hs[:, a, :,
                                                           t * 128 : (t + 1) * 128],
                                                        Ws[:, kk, :, :],
                                                        start=(a == 0 and pi == 0),
                                                        stop=(a == 15 and pi == 2),
                                                        perf_mode=PM.DoubleRow,
                                                    )
                                            if kh == 3:
                                                ti = blk * 8 + t
                                                st = pby.tile([128, 512], F32, tag="sty",
                                                              name=f"st_{blk}_{co}_{t}")
                                                nc.vector.scalar_tensor_tensor(
                                                    st, op2s[tq], 1.0 / 32.0,
                                                    r1_bf[:, ti, co * 512 : (co + 1) * 512],
                                                    ALU.mult, ALU.add,
                                                )
                                                nc.gpsimd.dma_start(
                                                    y_d[
                                                        blk * BT + t * 128 : blk * BT + (t + 1) * 128,
                                                        co * 512 : (co + 1) * 512,
                                                    ],
                                                    st,
                                                )

    nc.finalize()
    return nc


_NC_CACHE = {}


def _get_nc():
    if "nc" not in _NC_CACHE:
        _NC_CACHE["nc"] = build()
    return _NC_CACHE["nc"]


def _prep_weights(inputs):
    qkv_w = np.asarray(inputs["qkv_w"], dtype=np.float32)
    proj_w = np.asarray(inputs["proj_w"], dtype=np.float32)
    fc1_w = np.asarray(inputs["fc1_w"], dtype=np.float32)
    fc2_w = np.asarray(inputs["fc2_w"], dtype=np.float32)
    ln1_g = np.asarray(inputs["ln1_g"], dtype=np.float32)
    ln2_g = np.asarray(inputs["ln2_g"], dtype=np.float32)
    rel_pos_bias = np.asarray(inputs["rel_pos_bias"], dtype=np.float32)
    rel_pos_idx = np.asarray(inputs["rel_pos_idx"])

    wq = ln1_g[:, None] * qkv_w  # fold LN1 gamma (gamma == 1 asserted anyway)
    wf1 = ln2_g[:, None] * fc1_w

    # Q/K output-column permutation for split-d S layout:
    # feature (h, d) -> chunk 2*(h//4) + d//32, partition 32*(h%4) + d%32
    perm = np.zeros(C, dtype=np.int64)
    for h in range(H):
        for d in range(DH):
            ci = 2 * (h // 4) + (d // 32)
            p = 32 * (h % 4) + (d % 32)
            perm[ci * 128 + p] = h * DH + d
    wq_p = wq.copy()
    wq_p[:, 0:C] = wq[:, 0:C][:, perm]
    wq_p[:, C : 2 * C] = wq[:, C : 2 * C][:, perm]

    # [p, kk, i, col] = wq_p[(kk*2+i)*128 + p, col]
    wqkv8 = np.ascontiguousarray(
        wq_p.reshape(4, 2, 128, 3 * C).transpose(2, 0, 1, 3)
    ).astype(F8NP)
    wproj8 = np.ascontiguousarray(
        proj_w.reshape(4, 2, 128, C).transpose(2, 0, 1, 3)
    ).astype(F8NP)

    # bias table: b8tab[64*(h//8)+p, h%8, nk, i, q] = 8*Bm[q, nk*128+64*i+p, h]
    Bm = rel_pos_bias[rel_pos_idx].reshape(N, N, H)  # [q, k, h]
    BT_ = 8.0 * Bm.transpose(2, 1, 0)  # [h, k, q]
    btab = np.zeros((128, 8, 2, 2, N), dtype=np.float32)  # [p, h, i, nk, q]
    for h in range(H):
        hb = 64 * (h // 8)
        for nk in range(2):
            for i in range(2):
                btab[hb : hb + 64, h % 8, i, nk, :] = BT_[
                    h, nk * 128 + 64 * i : nk * 128 + 64 * i + 64, :
                ]
    btab8 = btab.reshape(128, 8, 2, 2 * N).astype(F8NP)

    # ident pair for bias preload
    id2 = np.zeros((128, 2, 128), dtype=np.float32)
    for p in range(128):
        for i in range(2):
            id2[p, i, 64 * i + (p % 64)] = 1.0
    id28 = id2.astype(F8NP)

    # fc1/fc2 hi/lo fp8 pairs, pre-scaled (8x / 32x) to keep the hi parts in
    # e4m3 normal range; the kernel undoes the scales at gelu / eviction.
    # DR layout [p, a, i, m] = W[(2a+i)*128 + p, m].
    w1s = 8.0 * wf1
    w1h = w1s.astype(F8NP)
    w1l = (w1s - w1h.astype(np.float32)).astype(F8NP)
    w2s = 32.0 * fc2_w
    w2h = w2s.astype(F8NP)
    w2l = (w2s - w2h.astype(np.float32)).astype(F8NP)
    lay1 = lambda w: np.ascontiguousarray(
        w.reshape(4, 2, 128, DFF).transpose(2, 0, 1, 3)
    )
    lay2 = lambda w: np.ascontiguousarray(
        w.reshape(16, 2, 128, C).transpose(2, 0, 1, 3)
    )
    return (wqkv8, wproj8, btab8, id28,
            lay1(w1h), lay1(w1l), lay2(w2h), lay2(w2l))


def kernel(**inputs):
    x = np.asarray(inputs["x"], dtype=np.float32)
    for k in ("qkv_b", "proj_b", "fc1_b", "fc2_b", "ln1_b", "ln2_b"):
        assert not np.any(np.asarray(inputs[k])), f"nonzero {k} unsupported"

    (wqkv8, wproj8, btab8, id28,
     wf1h8, wf1l8, wf2h8, wf2l8) = _prep_weights(inputs)

    nc = _get_nc()
    in_maps = []
    for c in range(NCORES):
        xs = np.ascontiguousarray(
            x[c * BLOC : (c + 1) * BLOC].reshape(TOK, C)
        ).astype(np.float32)
        mu = xs.mean(axis=1)
        var = xs.var(axis=1)
        xs = xs.astype(BFNP)
        rstd = 1.0 / np.sqrt(var + 1e-5)
        ms1 = np.stack([mu, rstd], axis=-1).reshape(16, 128, 2).transpose(1, 0, 2)
        in_maps.append(
            dict(
                x=xs,
                ms1=np.ascontiguousarray(ms1).astype(np.float32),
                wqkv=wqkv8,
                wproj=wproj8,
                btab=btab8,
                id2=id28,
                wfc1h=wf1h8,
                wfc1l=wf1l8,
                wfc2h=wf2h8,
                wfc2l=wf2l8,
            )
        )
    res = run_bass_kernel_spmd(nc, in_maps, core_ids=list(range(NCORES)))
    y = np.concatenate([res.results[c]["y"] for c in range(NCORES)], axis=0)
    return y.reshape(B, N, C).astype(np.float32)



# revision 28
# speedup vs baseline: 1.0170x; 1.0170x over previous
"""LITv1 transformer block on 8 TRN2 NeuronCores, data-parallel over batch.

v3 design:
- Attention phase in fp8e4 with DoubleRow matmuls (QKV, S, PV, proj):
  2 k-tiles per instruction at 0.5 cycles/row.
- Relative-position bias folded into the S PSUM accumulation via an fp8
  identity-pair preload matmul (8x bias table, exp scale 0.125, bias -4
  shift keeps exp outputs within fp8 range).
- Softmax denominator via ones-column in V (DR out [65, N]); per-head
  reciprocal + broadcast matmul + normalize multiply.
- LN1 mean/rstd computed host-side (x is an input); avoids Act sqrt
  table switches against exp.
- LN normalize outputs bf16; transposes done by the DMA xbar
  (dma_start_transpose), not the PE.
- MLP in bf16 (fp8 too lossy there); fc1 dff-major + fused gelu,
  fc2 token-major; residual stream r1 kept in SBUF as bf16.
"""
import sys
from contextlib import ExitStack

import numpy as np
import ml_dtypes

sys.path.insert(0, "/opt/trn_rl_repo")

import concourse.bass as bass  # noqa: E402
import concourse.mybir as mybir  # noqa: E402
import concourse.tile as tile  # noqa: E402
from concourse import bacc  # noqa: E402
from concourse.bass_utils import run_bass_kernel_spmd  # noqa: E402

F32 = mybir.dt.float32
BF16 = mybir.dt.bfloat16
F8 = mybir.dt.float8e4
AF = mybir.ActivationFunctionType
ALU = mybir.AluOpType
PM = mybir.MatmulPerfMode

B, N, C = 64, 256, 1024
H, DH = 16, 64
DFF = 4 * C
NCORES = 8
BLOC = B // NCORES          # 8 batches per core
TOK = BLOC * N              # 2048 tokens per core
KC = C // 128               # 8 contraction chunks
NB = 2                      # MLP token blocks
BT = TOK // NB              # 1024 tokens per MLP block

F8NP = ml_dtypes.float8_e4m3
BFNP = ml_dtypes.bfloat16


def build():
    nc = bacc.Bacc("TRN2")
    x_d = nc.dram_tensor("x", [TOK, C], BF16, kind="ExternalInput")
    ms1_d = nc.dram_tensor("ms1", [128, 16, 2], F32, kind="ExternalInput")
    wqkv_d = nc.dram_tensor("wqkv", [128, 4, 2, 3 * C], F8, kind="ExternalInput")
    wproj_d = nc.dram_tensor("wproj", [128, 4, 2, C], F8, kind="ExternalInput")
    btab_d = nc.dram_tensor("btab", [128, 8, 2, 2 * N], F8, kind="ExternalInput")
    wfc1h_d = nc.dram_tensor("wfc1h", [128, 4, 2, DFF], F8, kind="ExternalInput")
    wfc1l_d = nc.dram_tensor("wfc1l", [128, 4, 2, DFF], F8, kind="ExternalInput")
    wfc2h_d = nc.dram_tensor("wfc2h", [128, 16, 2, C], F8, kind="ExternalInput")
    wfc2l_d = nc.dram_tensor("wfc2l", [128, 16, 2, C], F8, kind="ExternalInput")
    y_d = nc.dram_tensor("y", [TOK, C], F32, kind="ExternalOutput")

    with tile.TileContext(nc) as tc:
        with ExitStack() as _es:
            _es.enter_context(nc.allow_low_precision(reason="bf16 residual/LN"))
            consts = _es.enter_context(tc.tile_pool(name="consts", bufs=1))
            # ident2[p, i, k] = 1 if k == 64*i + (p % 64): fp8 ident pair for
            # the bias preload (duplicated in both partition halves).
            id2 = consts.tile([128, 2, 128], F8)
            # build via iota trick: set with 128 tiny memsets would be slow;
            # use affine_select-free approach: write from DRAM instead.
            # (cheap: 32KB one-off DMA)
            id2_d = nc.dram_tensor("id2", [128, 2, 128], F8, kind="ExternalInput")
            nc.sync.dma_start(id2, id2_d[:])
            ones64_bf = consts.tile([1, 64], BF16)
            nc.vector.memset(ones64_bf, 1.0)
            ones8 = consts.tile([128, 2, 128], F8)
            nc.vector.memset(ones8, 1.0)
            eps_sb = consts.tile([128, 1], F32)
            nc.vector.memset(eps_sb, 1e-5)
            nbias = consts.tile([128, 1], F32)
            nc.vector.memset(nbias, -3.5)
            ms1 = consts.tile([128, 16, 2], F32)
            nc.sync.dma_start(ms1, ms1_d[:])
            mv2 = consts.tile([128, 16, 2], F32)
            rstd2 = consts.tile([128, 16], F32)
            srt = consts.tile([128, 16], F32)
            r1_bf = consts.tile([128, 16, C], BF16)
            pbw1 = _es.enter_context(tc.tile_pool(name="pbw1", bufs=2))
            pbw2 = _es.enter_context(tc.tile_pool(name="pbw2", bufs=4))
            pbt2 = _es.enter_context(tc.tile_pool(name="pbt2", bufs=1))
            pbx = _es.enter_context(tc.tile_pool(name="pbx", bufs=1))
            # shared tag: blk0's xn2T is dead once its hi/lo split is made
            # (phase A b4/b5), so blk1's transposes can reuse the buffer
            xn2Ts = [pbt2.tile([128, KC, BT], BF16, tag="xn2T",
                               name=f"xn2T{i}") for i in range(NB)]
            # hi/lo fp8 split of the transposed LN2 output, for 3-product
            # DoubleRow fc1 (Xh@(W1h+W1l) + Xl@W1h).  Shared tags: blk1
            # reuses blk0's buffers (its split is emitted after blk0's fc1
            # reads so the WAR dependency orders correctly).
            XhXl = [
                (pbx.tile([128, KC, BT], F8, tag="Xh", name=f"Xh{b_}"),
                 pbx.tile([128, KC, BT], F8, tag="Xl", name=f"Xl{b_}"))
                for b_ in range(NB)
            ]

            def emit_xsplit(blk, half, pool_eng):
                Xh, Xl = XhXl[blk]
                sl = slice(half * 512, (half + 1) * 512)
                if pool_eng:
                    nc.gpsimd.tensor_copy(Xh[:, :, sl], xn2Ts[blk][:, :, sl])
                    nc.gpsimd.tensor_sub(
                        Xl[:, :, sl], xn2Ts[blk][:, :, sl], Xh[:, :, sl]
                    )
                else:
                    nc.scalar.activation(
                        Xh[:, :, sl], xn2Ts[blk][:, :, sl], AF.Copy
                    )
                    nc.vector.tensor_sub(
                        Xl[:, :, sl], xn2Ts[blk][:, :, sl], Xh[:, :, sl]
                    )

            def emit_wf1(blk, out, s0, s1):
                for s in range(s0, s1):
                    wf1h = pbw1.tile([128, 4, 2, 512], F8, tag="wf1h",
                                     name=f"wf1h_{blk}_{s}")
                    wf1l = pbw1.tile([128, 4, 2, 512], F8, tag="wf1l",
                                     name=f"wf1l_{blk}_{s}")
                    nc.sync.dma_start(
                        wf1h, wfc1h_d[:, :, :, s * 512 : (s + 1) * 512]
                    )
                    nc.sync.dma_start(
                        wf1l, wfc1l_d[:, :, :, s * 512 : (s + 1) * 512]
                    )
                    out.append((wf1h, wf1l))

            mv2c = consts.tile([128, 16, 2], F32)

            def emit_rstd(lo, hi):
                # copy gates the sqrt block on the last producer of mv2[lo:hi]
                # so the scheduler cannot scatter the sqrts (and their Act
                # table loads) across earlier exp batches
                nc.vector.tensor_copy(mv2c[:, lo:hi, :], mv2[:, lo:hi, :])
                for ti in range(lo, hi):
                    nc.scalar.activation(
                        srt[:, ti : ti + 1], mv2c[:, ti, 1:2], AF.Sqrt,
                        bias=eps_sb, scale=1.0,
                    )
                nc.vector.reciprocal(rstd2[:, lo:hi], srt[:, lo:hi])
            wf1s = []

            # ---------------- Phase A: attention + proj ----------------
            with ExitStack() as es:
                tp = lambda nm, bufs, **kw: es.enter_context(tc.tile_pool(name=nm, bufs=bufs, **kw))
                paw = tp("paw", 1); pax = tp("pax", 3); pan = tp("pan", 1)
                pat = tp("pat", 1); pa8 = tp("pa8", 2); paq = tp("paq", 2)
                pav = tp("pav", 2); par = tp("par", 4); pao = tp("pao", 2)
                psQV = tp("psQV", 2, space="PSUM")
                psS = tp("psS", 2, space="PSUM"); psO = tp("psO", 2, space="PSUM")
                psDB = tp("psDB", 2, space="PSUM")
                wqkv_sb = paw.tile([128, 4, 2, 3 * C], F8)
                wproj_sb = paw.tile([128, 4, 2, C], F8)
                btab = paw.tile([128, 8, 2, 2 * N], F8)

                fronts = {}
                oalls = {}

                def emit_front(b):
                    t0 = b * N
                    # --- LN1 (host stats) -> xn bf16 -> DMA transpose ---
                    xt = pax.tile([128, 2, C], BF16, tag="x")
                    xnT = pat.tile([128, KC, N], BF16, tag="xnT")
                    for t in range(2):
                        ti = 2 * b + t
                        nc.sync.dma_start(
                            xt[:, t, :], x_d[t0 + t * 128 : t0 + (t + 1) * 128, :]
                        )
                        xn = pan.tile([128, C], BF16, tag="xn")
                        ln_eng = nc.vector if b == 0 else nc.gpsimd
                        ln_eng.tensor_scalar(
                            xn, xt[:, t, :], ms1[:, ti, 0:1], ms1[:, ti, 1:2],
                            ALU.subtract, ALU.mult,
                        )
                        nc.sync.dma_start_transpose(
                            xnT[:, :, t * 128 : (t + 1) * 128], xn
                        )
                    if b == 0:
                        # chunked weight DMAs: the first QKV matmuls
                        # only wait for the first 512-col chunk
                        for ci in range(4):
                            nc.sync.dma_start(
                                wqkv_sb[:, :, :, ci * 512 : (ci + 1) * 512],
                                wqkv_d[:, :, :, ci * 512 : (ci + 1) * 512],
                            )
                        nc.sync.dma_start(btab, btab_d[:])
                        nc.sync.dma_start(
                            wqkv_sb[:, :, :, 2 * C :], wqkv_d[:, :, :, 2 * C :]
                        )
                        nc.sync.dma_start(wproj_sb, wproj_d[:])
                    xnT8 = pa8.tile([128, KC, N], F8, tag="xnT8")
                    for t in range(2):
                        nc.gpsimd.tensor_copy(
                            xnT8[:, :, t * 128 : (t + 1) * 128],
                            xnT[:, :, t * 128 : (t + 1) * 128],
                        )

                    # --- QKV --- (b==0: split q/k over token halves so the
                    # first matmuls only wait on the first front half)
                    qkT8 = paq.tile([128, 2 * KC, N], F8, tag="qkT8")
                    tsplit = 2 if b == 0 else 1
                    for co in range(2 * KC):
                        qv = psQV.tile([128, 512], F32, tag="qv")
                        qp = qv[:, 0:N]
                        for ts in range(tsplit):
                            tsl = slice(ts * (N // tsplit),
                                        (ts + 1) * (N // tsplit))
                            for kk in range(4):
                                nc.tensor.matmul(
                                    qp[:, tsl],
                                    wqkv_sb[:, kk, :, co * 128 : (co + 1) * 128],
                                    xnT8.rearrange("p (a i) n -> p a i n", i=2)[
                                        :, kk, :, tsl
                                    ],
                                    start=(kk == 0),
                                    stop=(kk == 3),
                                    perf_mode=PM.DoubleRow,
                                )
                        if co % 2 == 0:
                            nc.vector.tensor_copy(qkT8[:, co, :], qp)
                        else:
                            nc.scalar.copy(qkT8[:, co, :], qp)
                    v8 = pav.tile([128, 2, H, DH], F8, tag="v8")
                    for t in range(2):
                        for vc in range(2):
                            vp = psQV.tile([128, 512], F32, tag="qv")
                            for kk in range(4):
                                nc.tensor.matmul(
                                    vp,
                                    xnT8.rearrange("p (a i) n -> p a i n", i=2)[
                            :, kk, :, t * 128 : (t + 1) * 128
                                    ],
                                    wqkv_sb[:, kk, :, 2 * C + vc * 512 : 2 * C + (vc + 1) * 512],
                                    start=(kk == 0),
                                    stop=(kk == 3),
                                    perf_mode=PM.DoubleRow,
                                )
                            nc.vector.tensor_copy(
                                v8[:, t, vc * 8 : (vc + 1) * 8, :],
                                vp.rearrange("p (h d) -> p h d", h=8),
                            )

                    fronts[b] = (xt, qkT8, v8)

                def emit_attn(b):
                    xt, qkT8, v8 = fronts[b]
                    # --- attention, head-pipelined ---
                    sps, p8s, ops, rds, dbs = {}, {}, {}, {}, {}
                    oall8 = pao.tile([128, KC, N], F8, tag="oall8",
                                     name=f"oall8_{b}")
                    oalls[b] = oall8

                    def emit_S(h):
                        pb = 32 * (h % 4)
                        cp = 2 * (h // 4)
                        hb = 64 * (h // 8)
                        p8 = par.tile([128, 2, N], F8, tag="p8", name=f"p8_{b}_{h}")
                        sp = psS.tile([128, 2, N], F32, tag="sp", name=f"sp_{b}_{h}")
                        spf = sp.rearrange("p a n -> p (a n)")
                        nc.tensor.matmul(
                            spf,
                            id2[hb : hb + 64, :, :],
                            btab[hb : hb + 64, h % 8, :, :],
                            start=True,
                            stop=False,
                            perf_mode=PM.DoubleRow,
                            skip_group_check=True,
                            tile_position=(hb, 0),
                        )
                        for nk in range(2):
                            nc.tensor.matmul(
                                sp[:, nk, :],
                                qkT8[pb : pb + 32, KC + cp : KC + cp + 2,
                                     nk * 128 : (nk + 1) * 128],
                                qkT8[pb : pb + 32, cp : cp + 2, :],
                                start=False,
                                stop=(nk == 1),
                                perf_mode=PM.DoubleRow,
                                skip_group_check=True,
                                tile_position=(pb, 0),
                            )
                        nc.scalar.activation(
                            p8.rearrange("p a n -> p (a n)"), spf, AF.Exp,
                            bias=nbias, scale=0.125,
                        )
                        p8s[h] = p8

                    def emit_PV(h):
                        op = psO.tile([64, N], F32, tag="op", name=f"op_{b}_{h}")
                        nc.tensor.matmul(
                            op,
                            v8[:, :, h, :],
                            p8s[h][:],
                            start=True,
                            stop=True,
                            perf_mode=PM.DoubleRow,
                        )
                        db = psDB.tile([64, 2, N], F32, tag="db", name=f"db_{b}_{h}")
                        nc.tensor.matmul(
                            db[0:1, 0, :], ones8[:, :, 0:1], p8s[h][:],
                            start=True, stop=True, perf_mode=PM.DoubleRow,
                        )
                        rd = par.tile([1, N], BF16, tag="rd", name=f"rd_{b}_{h}")
                        nc.vector.reciprocal(rd, db[0:1, 0, :])
                        ops[h] = op
                        dbs[h] = db
                        rds[h] = rd

                    def emit_norm(h):
                        bc = dbs[h][:, 1, :]
                        nc.tensor.matmul(
                            bc, ones64_bf, rds[h], start=True, stop=True
                        )
                        bc_sb = par.tile([64, N], BF16, tag="bcs", name=f"bcs_{b}_{h}")
                        nc.scalar.copy(bc_sb, bc)
                        nc.vector.tensor_mul(
                            oall8[64 * (h % 2) : 64 * (h % 2) + 64, h // 2, :],
                            ops[h][:],
                            bc_sb,
                        )

                    for h in range(H):
                        emit_S(h)
                        if h >= 1:
                            emit_PV(h - 1)
                        if h >= 2:
                            emit_norm(h - 2)
                    emit_PV(H - 1)
                    emit_norm(H - 2)
                    emit_norm(H - 1)

                def emit_proj(b):
                    xt, qkT8, v8 = fronts[b]
                    oall8 = oalls[b]
                    t0 = b * N
                    # --- proj + residual -> r1_bf, LN2 stats ---
                    for t in range(2):
                        ti = 2 * b + t
                        stats = pan.tile([128, 2, 6], F32, tag="st2")
                        for co in range(2):
                            pp = psQV.tile([128, 512], F32, tag="qv")
                            for kk in range(4):
                                nc.tensor.matmul(
                                    pp,
                                    oall8[:, 2 * kk : 2 * kk + 2,
                              t * 128 : (t + 1) * 128],
                                    wproj_sb[:, kk, :, co * 512 : (co + 1) * 512],
                                    start=(kk == 0),
                                    stop=(kk == 3),
                                    perf_mode=PM.DoubleRow,
                                )
                            nc.vector.tensor_add(
                                r1_bf[:, ti, co * 512 : (co + 1) * 512],
                                pp,
                                xt[:, t, co * 512 : (co + 1) * 512],
                            )
                            nc.vector.bn_stats(
                                stats[:, co, :],
                                r1_bf[:, ti, co * 512 : (co + 1) * 512],
                            )
                        nc.vector.bn_aggr(mv2[:, ti, :], stats)

                    if b == 3:
                        emit_rstd(0, 8)
                        for t in range(BT // 128):
                            xn2 = pan.tile([128, C], BF16, tag="xn2e",
                               name=f"xn2e_{t}")
                            nc.vector.tensor_scalar(
                                xn2, r1_bf[:, t, :], mv2[:, t, 0:1],
                                rstd2[:, t : t + 1], ALU.subtract, ALU.mult,
                            )
                            nc.sync.dma_start_transpose(
                                xn2Ts[0][:, :, t * 128 : (t + 1) * 128], xn2
                            )



                emit_front(0)
                for b in range(BLOC):
                    emit_attn(b)
                    if b + 1 < BLOC:
                        emit_front(b + 1)
                    emit_proj(b)
                    del fronts[b], oalls[b]

            # ------- Phase B: MLP, fp8 DoubleRow with hi/lo 3-product -------
            # fc1: Xh@(W1h+W1l) + Xl@W1h at 8x weight scale (undone in the
            # gelu scale); fc2: Hh@(W2h+W2l) + Hl@W2h at 32x (undone in the
            # fused eviction).  rstd2 for all tiles first (single sqrt-table
            # block on Act).
            with (
                tc.tile_pool(name="pbs", bufs=1) as pbs,
                tc.tile_pool(name="psF1", bufs=4, space="PSUM") as psF1,
                tc.tile_pool(name="psF2", bufs=1, space="PSUM") as psF2,
            ):
                # blk0's first fc1 weight slices before anything else hits
                # the SP queue (stall-free: within pbw1's rotation depth)
                wf1s.clear()
                emit_wf1(0, wf1s, 0, 2)
                emit_rstd(8, 16)
                # blk0 hi/lo split first: it reads xn2Ts[0], whose buffer the
                # blk1 transposes below will reuse (shared tag)
                emit_xsplit(0, 0, pool_eng=False)
                emit_xsplit(0, 1, pool_eng=False)
                for t in range(BT // 128):
                    ti = 8 + t
                    xn2 = pbs.tile([128, C], BF16, tag="xn2p", name=f"xn2p_{t}",
                                   bufs=2)
                    nc.vector.tensor_scalar(
                        xn2, r1_bf[:, ti, :], mv2[:, ti, 0:1],
                        rstd2[:, ti : ti + 1], ALU.subtract, ALU.mult,
                    )
                    nc.sync.dma_start_transpose(
                        xn2Ts[1][:, :, t * 128 : (t + 1) * 128], xn2
                    )
                for blk in range(NB):
                    if blk == 1:
                        emit_xsplit(1, 0, pool_eng=False)
                        emit_xsplit(1, 1, pool_eng=False)
                    with ExitStack() as esb:
                        tpb = lambda nm, bufs, **kw: esb.enter_context(tc.tile_pool(name=nm, bufs=bufs, **kw))
                        pbh = tpb("pbh", 1); pbg = tpb("pbg", 3)
                        Xh, Xl = XhXl[blk]
                        XhV = Xh.rearrange("p (a i) n -> p a i n", i=2)
                        XlV = Xl.rearrange("p (a i) n -> p a i n", i=2)
                        Hh = pbh.tile([128, 32, BT], F8, tag="Hh",
                                      name=f"Hh_{blk}")
                        Hl = pbh.tile([128, 32, BT], F8, tag="Hl",
                                      name=f"Hl_{blk}")
                        HhV = Hh.rearrange("p (a i) n -> p a i n", i=2)
                        HlV = Hl.rearrange("p (a i) n -> p a i n", i=2)
                        if blk > 0:
                            wf1s.clear()
                            emit_wf1(blk, wf1s, 0, 2)
                        for s in range(8):
                            # depth-2 JIT prefetch: never a dep-stalled DMA
                            # parked at the SP queue head for long
                            if s + 2 <= 7:
                                emit_wf1(blk, wf1s, s + 2, s + 3)
                            wf1h, wf1l = wf1s[s]
                            for dc in range(4):
                                ch = s * 4 + dc
                                for th in range(BT // 512):
                                    fp = psF1.tile([128, 512], F32, tag="fp")
                                    k = 0
                                    for W, X in ((wf1h, XhV), (wf1l, XhV),
                                                 (wf1h, XlV)):
                                        for a in range(4):
                                            nc.tensor.matmul(
                                                fp,
                                                W[:, a, :, dc * 128 : (dc + 1) * 128],
                                                X[:, a, :, th * 512 : (th + 1) * 512],
                                                start=(k == 0),
                                                stop=(k == 11),
                                                perf_mode=PM.DoubleRow,
                                            )
                                            k += 1
                                    tsl = slice(th * 512, (th + 1) * 512)
                                    nc.scalar.activation(
                                        Hh[:, ch, tsl], fp,
                                        AF.Gelu_apprx_tanh, scale=0.125,
                                    )
                                    hb = pbg.tile([128, 512], BF16, tag="hb")
                                    nc.scalar.activation(
                                        hb, fp, AF.Gelu_apprx_tanh, scale=0.125,
                                    )
                                    nc.gpsimd.tensor_sub(
                                        Hl[:, ch, tsl], hb, Hh[:, ch, tsl]
                                    )

                        # fc2 token-major + residual -> y (2 token groups
                        # of 4 so psF2 fits in 4 banks alongside psF1)
                        with ExitStack() as esc:
                            tpc = lambda nm, bufs, **kw: esc.enter_context(tc.tile_pool(name=nm, bufs=bufs, **kw))
                            pby = tpc("pby", 2)
                            for co in range(2):
                                wf2s = []
                                for kh in range(4):
                                    w2h = pbw2.tile([128, 4, 2, 512], F8,
                                                    tag="w2h",
                                                    name=f"w2h_{blk}_{co}_{kh}")
                                    w2l = pbw2.tile([128, 4, 2, 512], F8,
                                                    tag="w2l",
                                                    name=f"w2l_{blk}_{co}_{kh}")
                                    nc.sync.dma_start(
                                        w2h,
                                        wfc2h_d[:, kh * 4 : (kh + 1) * 4, :,
                                                co * 512 : (co + 1) * 512],
                                    )
                                    nc.sync.dma_start(
                                        w2l,
                                        wfc2l_d[:, kh * 4 : (kh + 1) * 4, :,
                                                co * 512 : (co + 1) * 512],
                                    )
                                    wf2s.append((w2h, w2l))
                                for tg in range(2):
                                    op2s = [
                                        psF2.tile([128, 512], F32, tag=f"op2_{tq}",
                                                  name=f"op2_{blk}_{co}_{tg}_{tq}")
                                        for tq in range(4)
                                    ]
                                    for kh in range(4):
                                        w2h, w2l = wf2s[kh]
                                        for tq in range(4):
                                            t = tg * 4 + tq
                                            for kk in range(4):
                                                a = kh * 4 + kk
                                                for pi, (Hs, Ws) in enumerate(
                                                    ((HhV, w2h), (HhV, w2l),
                                                     (HlV, w2h))
                                                ):
                                                    nc.tensor.matmul(
                                                        op2s[tq],
                                                        Hs[:, a, :,
                                                           t * 128 : (t + 1) * 128],
                                                        Ws[:, kk, :, :],
                                                        start=(a == 0 and pi == 0),
                                                        stop=(a == 15 and pi == 2),
                                                        perf_mode=PM.DoubleRow,
                                                    )
                                            if kh == 3:
                                                ti = blk * 8 + t
                                                st = pby.tile([128, 512], F32, tag="sty",
                                                              name=f"st_{blk}_{co}_{t}")
                                                nc.vector.scalar_tensor_tensor(
                                                    st, op2s[tq], 1.0 / 32.0,
                                                    r1_bf[:, ti, co * 512 : (co + 1) * 512],
                                                    ALU.mult, ALU.add,
                                                )
                                                nc.gpsimd.dma_start(
                                                    y_d[
                                                        blk * BT + t * 128 : blk * BT + (t + 1) * 128,
                                                        co * 512 : (co + 1) * 512,
                                                    ],
                                                    st,
                                                )

    nc.finalize()
    return nc


_NC_CACHE = {}


def _get_nc():
    if "nc" not in _NC_CACHE:
        _NC_CACHE["nc"] = build()
    return _NC_CACHE["nc"]


def _prep_weights(inputs):
    qkv_w = np.asarray(inputs["qkv_w"], dtype=np.float32)
    proj_w = np.asarray(inputs["proj_w"], dtype=np.float32)
    fc1_w = np.asarray(inputs["fc1_w"], dtype=np.float32)
    fc2_w = np.asarray(inputs["fc2_w"], dtype=np.float32)
    ln1_g = np.asarray(inputs["ln1_g"], dtype=np.float32)
    ln2_g = np.asarray(inputs["ln2_g"], dtype=np.float32)
    rel_pos_bias = np.asarray(inputs["rel_pos_bias"], dtype=np.float32)
    rel_pos_idx = np.asarray(inputs["rel_pos_idx"])

    wq = ln1_g[:, None] * qkv_w  # fold LN1 gamma (gamma == 1 asserted anyway)
    wf1 = ln2_g[:, None] * fc1_w

    # Q/K output-column permutation for split-d S layout:
    # feature (h, d) -> chunk 2*(h//4) + d//32, partition 32*(h%4) + d%32
    perm = np.zeros(C, dtype=np.int64)
    for h in range(H):
        for d in range(DH):
            ci = 2 * (h // 4) + (d // 32)
            p = 32 * (h % 4) + (d % 32)
            perm[ci * 128 + p] = h * DH + d
    wq_p = wq.copy()
    wq_p[:, 0:C] = wq[:, 0:C][:, perm]
    wq_p[:, C : 2 * C] = wq[:, C : 2 * C][:, perm]

    # [p, kk, i, col] = wq_p[(kk*2+i)*128 + p, col]
    wqkv8 = np.ascontiguousarray(
        wq_p.reshape(4, 2, 128, 3 * C).transpose(2, 0, 1, 3)
    ).astype(F8NP)
    wproj8 = np.ascontiguousarray(
        proj_w.reshape(4, 2, 128, C).transpose(2, 0, 1, 3)
    ).astype(F8NP)

    # bias table: b8tab[64*(h//8)+p, h%8, nk, i, q] = 8*Bm[q, nk*128+64*i+p, h]
    Bm = rel_pos_bias[rel_pos_idx].reshape(N, N, H)  # [q, k, h]
    BT_ = 8.0 * Bm.transpose(2, 1, 0)  # [h, k, q]
    btab = np.zeros((128, 8, 2, 2, N), dtype=np.float32)  # [p, h, i, nk, q]
    for h in range(H):
        hb = 64 * (h // 8)
        for nk in range(2):
            for i in range(2):
                btab[hb : hb + 64, h % 8, i, nk, :] = BT_[
                    h, nk * 128 + 64 * i : nk * 128 + 64 * i + 64, :
                ]
    btab8 = btab.reshape(128, 8, 2, 2 * N).astype(F8NP)

    # ident pair for bias preload
    id2 = np.zeros((128, 2, 128), dtype=np.float32)
    for p in range(128):
        for i in range(2):
            id2[p, i, 64 * i + (p % 64)] = 1.0
    id28 = id2.astype(F8NP)

    # fc1/fc2 hi/lo fp8 pairs, pre-scaled (8x / 32x) to keep the hi parts in
    # e4m3 normal range; the kernel undoes the scales at gelu / eviction.
    # DR layout [p, a, i, m] = W[(2a+i)*128 + p, m].
    w1s = 8.0 * wf1
    w1h = w1s.astype(F8NP)
    w1l = (w1s - w1h.astype(np.float32)).astype(F8NP)
    w2s = 32.0 * fc2_w
    w2h = w2s.astype(F8NP)
    w2l = (w2s - w2h.astype(np.float32)).astype(F8NP)
    lay1 = lambda w: np.ascontiguousarray(
        w.reshape(4, 2, 128, DFF).transpose(2, 0, 1, 3)
    )
    lay2 = lambda w: np.ascontiguousarray(
        w.reshape(16, 2, 128, C).transpose(2, 0, 1, 3)
    )
    return (wqkv8, wproj8, btab8, id28,
            lay1(w1h), lay1(w1l), lay2(w2h), lay2(w2l))


def kernel(**inputs):
    x = np.asarray(inputs["x"], dtype=np.float32)
    for k in ("qkv_b", "proj_b", "fc1_b", "fc2_b", "ln1_b", "ln2_b"):
        assert not np.any(np.asarray(inputs[k])), f"nonzero {k} unsupported"

    (wqkv8, wproj8, btab8, id28,
     wf1h8, wf1l8, wf2h8, wf2l8) = _prep_weights(inputs)

    nc = _get_nc()
    in_maps = []
    for c in range(NCORES):
        xs = np.ascontiguousarray(
            x[c * BLOC : (c + 1) * BLOC].reshape(TOK, C)
        ).astype(np.float32)
        mu = xs.mean(axis=1)
        var = xs.var(axis=1)
        xs = xs.astype(BFNP)
        rstd = 1.0 / np.sqrt(var + 1e-5)
        ms1 = np.stack([mu, rstd], axis=-1).reshape(16, 128, 2).transpose(1, 0, 2)
        in_maps.append(
            dict(
                x=xs,
                ms1=np.ascontiguousarray(ms1).astype(np.float32),
                wqkv=wqkv8,
                wproj=wproj8,
                btab=btab8,
                id2=id28,
                wfc1h=wf1h8,
                wfc1l=wf1l8,
                wfc2h=wf2h8,
                wfc2l=wf2l8,
            )
        )
    res = run_bass_kernel_spmd(nc, in_maps, core_ids=list(range(NCORES)))
    y = np.concatenate([res.results[c]["y"] for c in range(NCORES)], axis=0)
    return y.reshape(B, N, C).astype(np.float32)

